# revision 4
# baseline (speedup 1.0000x reference)
"""Trainium2 Bass kernel for nn_Butterfly (batch=32768, 1024-dim, 10-stage untied
butterfly + bias). Data-parallel over batch across 8 cores, 4096 rows/core.

Feature-major design: the host transposes x to xT [1024, 4096] fp16 per core, so
no on-device transposes are needed. Stages 0-7 of the butterfly never cross
256-element blocks, so they fold (host-side, fp64) into a 256-block-diagonal
matrix W. Stages 8 and 9 (strides 256/512) are factored per lane as
B = U @ D with U unit-diagonal ([[1, b/d], [c/a, 1]]) and D = diag(a, d); both
D's commute into W's output columns. On device, per [1024f, 1024b] stripe:

  ya_t  = W' x_blk       16 matmuls (K=128, N=512) -> fp32 PSUM, 8 tiles
  e_t   = ya_t + beta_t  ACT evacuation PSUM->SBUF fp16, one per tile; the
                         output bias rides in beta (two pairwise 2x2 solves)
  z_t   = u8'_t * e_{t^2} + e_t    8 DVE scalar_tensor_tensor ops (stage 8)
  out_t = u9_t  * z_{t^4} + z_t    8 DVE scalar_tensor_tensor ops (stage 9)

All DVE ops are fp16 SBUF->SBUF (2x mode); ACT ops are the only PSUM readers.
Output is outT [1024, 4096] fp16; host transposes back and upcasts. Measured
fp16 numerics: ~5e-4 L2 (tolerance 2e-2). All I/O fp16 => 16.8 MB/core HBM
traffic, near the ~358 GB/s memory roofline.
"""

import numpy as np

import concourse.mybir as mybir
import concourse.tile as tile
from concourse import bacc
from concourse.bass_utils import run_bass_kernel_spmd

F32 = mybir.dt.float32
F16 = mybir.dt.float16

BATCH = 32768
NF = 1024
N_CORES = 8
BPC = BATCH // N_CORES      # 4096 batch rows per core
FD = 1024                   # stripe width (batch cols per stripe)
NSTRIPES = BPC // FD

S8 = [2, 3, 0, 1, 6, 7, 4, 5]   # stage-8 partner (tile XOR 2)
S9 = [4, 5, 6, 7, 0, 1, 2, 3]   # stage-9 partner (tile XOR 4)

SCAL_DT = F16               # dtype of STT per-partition scalars


def _butterfly_parts(twiddle, bias):
    """Fold stages 0-7 into W' (256-block-diag, D8*D9-scaled columns); return
    stage 8/9 unit off-diagonal coeffs and bias-folded evacuation offsets."""
    t = np.asarray(twiddle, dtype=np.float64)[0]     # [10, 512, 2, 2]
    x = np.eye(NF)
    for idx in range(8):
        stride = 1 << idx
        g = NF // (2 * stride)
        tt = t[idx].reshape(g, stride, 2, 2)
        xr = x.reshape(-1, g, 2, stride)
        x = np.einsum('gkij,bgjk->bgik', tt, xr).reshape(-1, NF)
    W = x                                            # [k_in, f_out]

    t8 = t[8].reshape(2, 256, 2, 2)
    t9 = t[9]                                        # [512, 2, 2]
    u8 = np.zeros((8, 128)); D8 = np.zeros((8, 128))
    for tt_ in range(8):
        g, k1 = tt_ // 4, tt_ % 4
        if k1 < 2:
            B = t8[g, k1 * 128 + np.arange(128)]
            u8[tt_] = B[:, 0, 1] / B[:, 1, 1]
            D8[tt_] = B[:, 0, 0]
        else:
            B = t8[g, (k1 - 2) * 128 + np.arange(128)]
            u8[tt_] = B[:, 1, 0] / B[:, 0, 0]
            D8[tt_] = B[:, 1, 1]
    u9 = np.zeros((8, 128)); D9 = np.zeros((8, 128))
    for tt_ in range(8):
        if tt_ < 4:
            B = t9[tt_ * 128 + np.arange(128)]
            u9[tt_] = B[:, 0, 1] / B[:, 1, 1]
            D9[tt_] = B[:, 0, 0]
        else:
            B = t9[(tt_ - 4) * 128 + np.arange(128)]
            u9[tt_] = B[:, 1, 0] / B[:, 0, 0]
            D9[tt_] = B[:, 1, 1]
    u8p = u8 * D9 / np.array(D9)[S8]                 # commute D9 through U8
    Wp = W * (D8 * D9).reshape(-1)[None, :]

    # bias -> evacuation offsets: (I + U9 P9)(I + U8' P8) beta = bias,
    # solved as two pairwise 2x2 systems per lane.
    bias_t = np.asarray(bias, dtype=np.float64).reshape(8, 128)
    dz = np.zeros((8, 128))
    for tt_ in range(4):
        e, f = u9[tt_], u9[tt_ + 4]
        det = 1.0 - e * f
        dz[tt_] = (bias_t[tt_] - e * bias_t[tt_ + 4]) / det
        dz[tt_ + 4] = (bias_t[tt_ + 4] - f * bias_t[tt_]) / det
    beta = np.zeros((8, 128))
    for tt_ in (0, 1, 4, 5):
        p = S8[tt_]
        e, f = u8p[tt_], u8p[p]
        det = 1.0 - e * f
        beta[tt_] = (dz[tt_] - e * dz[p]) / det
        beta[p] = (dz[p] - f * dz[tt_]) / det
    return Wp, u8p, u9, beta


def _build(repeat=1):
    nc = bacc.Bacc(None, target_bir_lowering=False)
    x_d = nc.dram_tensor("x", [NF, BPC], F16, kind="ExternalInput")
    w_d = nc.dram_tensor("w", [128, 16 * 128], F16, kind="ExternalInput")
    scal_d = nc.dram_tensor("scal", [128, 16], SCAL_DT, kind="ExternalInput")
    beta_d = nc.dram_tensor("beta", [128, 8], F32, kind="ExternalInput")
    out_d = nc.dram_tensor("out", [NF, BPC], F16, kind="ExternalOutput")

    import contextlib
    with tile.TileContext(nc) as tc:
        with (
            tc.tile_pool(name="const", bufs=1) as cpool,
            tc.tile_pool(name="xp", bufs=2 * 8) as xpool,
            tc.tile_pool(name="ep", bufs=10) as epool,
            tc.tile_pool(name="zp", bufs=10) as zpool,
            tc.tile_pool(name="op", bufs=10) as opool,
            tc.tile_pool(name="psum", bufs=4, space="PSUM") as ppool,
        ):
            w_sb = cpool.tile([128, 16 * 128], F16)
            nc.sync.dma_start(out=w_sb[:], in_=w_d[:])
            scal_sb = cpool.tile([128, 16], SCAL_DT)
            nc.sync.dma_start(out=scal_sb[:], in_=scal_d[:])
            beta_sb = cpool.tile([128, 8], F32)
            nc.sync.dma_start(out=beta_sb[:], in_=beta_d[:])

            loop_cm = (
                tc.For_i(0, repeat, 1, hint_engines=(mybir.EngineType.PE,))
                if repeat > 1
                else contextlib.nullcontext()
            )
            with loop_cm:
                body(nc, tc, xpool, epool, zpool, opool, ppool,
                     w_sb, scal_sb, beta_sb, x_d, out_d)
    nc.compile()
    return nc


def body(nc, tc, xpool, epool, zpool, opool, ppool, w_sb, scal_sb, beta_sb,
         x_d, out_d):
    MUL = mybir.AluOpType.mult
    ADD = mybir.AluOpType.add
    IDENT = mybir.ActivationFunctionType.Identity
    for s in range(NSTRIPES):
        b0 = s * FD
        xk = []
        for kt in range(8):
            xt = xpool.tile([128, FD], F16, tag="x")
            nc.sync.dma_start(
                out=xt[:], in_=x_d[kt * 128:(kt + 1) * 128, b0:b0 + FD]
            )
            xk.append(xt)

        z = [None] * 8
        # group order so that stage-9 pairs (0,4)/(2,6) complete early
        for (tl, th) in ((0, 2), (4, 6), (1, 3), (5, 7)):
            e = {}
            for t_ in (tl, th):
                blk = t_ // 2
                pt = ppool.tile([128, FD], F32, tag="ya")
                for nb in range(FD // 512):
                    for j in range(2):
                        nc.tensor.matmul(
                            pt[:, nb * 512:(nb + 1) * 512],
                            w_sb[:, (t_ * 2 + j) * 128:(t_ * 2 + j + 1) * 128],
                            xk[2 * blk + j][:, nb * 512:(nb + 1) * 512],
                            start=(j == 0),
                            stop=(j == 1),
                        )
                et = epool.tile([128, FD], F16, tag="e")
                nc.scalar.activation(
                    out=et[:], in_=pt[:], func=IDENT,
                    bias=beta_sb[:, t_:t_ + 1],
                )
                e[t_] = et
            # stage 8: z_t = u8'_t * e_partner + e_t  (all SBUF fp16)
            for t_ in (tl, th):
                zt = zpool.tile([128, FD], F16, tag="z")
                nc.vector.scalar_tensor_tensor(
                    out=zt[:], in0=e[S8[t_]][:], scalar=scal_sb[:, t_:t_ + 1],
                    in1=e[t_][:], op0=MUL, op1=ADD,
                )
                z[t_] = zt
            # stage 9 for any pair whose two z's are now ready
            for t_ in (tl, th):
                p9 = S9[t_]
                if z[p9] is not None:
                    for a, b in ((t_, p9), (p9, t_)):
                        ot = opool.tile([128, FD], F16, tag="o")
                        nc.vector.scalar_tensor_tensor(
                            out=ot[:], in0=z[b][:],
                            scalar=scal_sb[:, 8 + a:8 + a + 1],
                            in1=z[a][:], op0=MUL, op1=ADD,
                        )
                        nc.sync.dma_start(
                            out=out_d[a * 128:(a + 1) * 128, b0:b0 + FD],
                            in_=ot[:],
                        )


_nc_cache = {}


def _get_nc(repeat=1):
    if repeat not in _nc_cache:
        _nc_cache[repeat] = _build(repeat)
    return _nc_cache[repeat]


def _prepare_inputs(x, twiddle, bias):
    x = np.asarray(x, dtype=np.float32)
    twiddle = np.asarray(twiddle, dtype=np.float32)
    bias = np.asarray(bias, dtype=np.float32)
    Wp, u8p, u9, beta = _butterfly_parts(twiddle, bias)

    w_host = np.zeros((128, 16 * 128), dtype=np.float16)
    for t_ in range(8):
        blk = t_ // 2
        for j in range(2):
            w_host[:, (t_ * 2 + j) * 128:(t_ * 2 + j + 1) * 128] = (
                Wp[(2 * blk + j) * 128:(2 * blk + j + 1) * 128,
                   t_ * 128:(t_ + 1) * 128].astype(np.float16)
            )
    scal = np.zeros((128, 16), dtype=mybir.dt.np(SCAL_DT))
    for t_ in range(8):
        scal[:, t_] = u8p[t_]
        scal[:, 8 + t_] = u9[t_]
    beta_host = np.ascontiguousarray(beta.T.astype(np.float32))  # [128, 8]

    xh = x.astype(np.float16)
    return [
        {
            "x": np.ascontiguousarray(xh[i * BPC:(i + 1) * BPC].T),
            "w": w_host,
            "scal": scal,
            "beta": beta_host,
        }
        for i in range(N_CORES)
    ]


def _run(in_maps, repeat=1, **kwargs):
    nc = _get_nc(repeat)
    return run_bass_kernel_spmd(nc, in_maps, core_ids=list(range(N_CORES)), **kwargs)


def kernel(x, twiddle, bias):
    in_maps = _prepare_inputs(x, twiddle, bias)
    res = _run(in_maps)
    out = np.empty((BATCH, NF), dtype=np.float32)
    for i in range(N_CORES):
        out[i * BPC:(i + 1) * BPC] = res.results[i]["out"].T
    return out
